# revision 2
# baseline (speedup 1.0000x reference)
"""Trainium2 Bass kernel for nn_BPFeedForward (per-element-type MLP, moe_routing).

Strategy: data-parallel over atoms. Each of the 8 cores gets 1/8 of each
element type's fingerprint rows (transposed to [D, m] on host), runs the
4-layer MLP with feature-on-partition / atoms-on-free layout:

    H_T[h, m] = tanh(W.T @ X_T + b)   (matmul lhsT = W as stored, rhs = X_T)

The kernel is ACT(scalar-engine)-bound: 768 tanh evaluations per atom at
1 col/cycle/128-lanes @ 1.2 GHz is ~125us per core. So the structure is
chosen to minimize ACT per-instruction overhead (~185ns each): PSUM is
split into two [128, 2048] half-arenas (4 banks each) and every tanh
activation drains a full 2048-wide psum region in ONE instruction.

Per span (2048 atoms of one element) the psum slot rotation is
  L0h0, L0h1, L1h0, L1h1, L2h0, L2h1, Lout
alternating the two half-arenas; PE fills one half while ACT drains the
other. Matmuls are kt-outer / 512-chunk-inner so the stationary weight
switches only every 4 matmuls (LDWEIGHTS hides under streaming).
Final layer: wo as lhsT [128,1] accumulating kt0+kt1 into psum [1, w];
DVE copies to SBUF, DMA to DRAM. Host applies the reference's
segment-sum routing in numpy.
"""

import sys

if "/opt/trn_rl_repo" not in sys.path:
    sys.path.insert(0, "/opt/trn_rl_repo")

import numpy as np

N_CORES = 8
E = 4
N_ATOMS = 200000
M_TOTAL = N_ATOMS // E          # 50000 atoms per element type
MPC = M_TOTAL // N_CORES        # 6250 atoms per element per core
D = 128
H = 256
CHUNK = 512
SUPER = 2048                    # superchunk width (one ACT drain unit)
MP = MPC

MODE = "f32r"                   # "f32r" or "bf16"

_COMPILED = {}

# spans per element: all matmul chunks must be >= 256 cols for f32r full
# rate, so split 6250 = 2048 + 2048 + 1898 + 256 (1898 -> 512*3 + 362).
SPANS = []
_pos = 0
while _pos < MPC:
    _rem = MPC - _pos
    if _rem >= SUPER + 256 or _rem == SUPER:
        _w = SUPER
    elif _rem > SUPER:
        _w = _rem - 256
    else:
        _w = _rem
    SPANS.append((_pos, _w))
    _pos += _w


def _np_dtype(mode):
    if mode == "bf16":
        import ml_dtypes
        return ml_dtypes.bfloat16
    return np.float32


def _build_program(reps: int = 1, mode: str = MODE):
    import concourse.bass as bass  # noqa: F401
    import concourse.mybir as mybir
    import concourse.tile as tile
    from concourse import bacc

    F32 = mybir.dt.float32
    MMDT = mybir.dt.float32r if mode == "f32r" else mybir.dt.bfloat16
    Tanh = mybir.ActivationFunctionType.Tanh

    nc = bacc.Bacc(None, target_bir_lowering=False, debug=False)

    xt = nc.dram_tensor("xt", [E, D, MP], MMDT, kind="ExternalInput")
    w0 = nc.dram_tensor("w0", [128, E, H], MMDT, kind="ExternalInput")
    w1 = nc.dram_tensor("w1", [128, E, 2, H], MMDT, kind="ExternalInput")
    w2 = nc.dram_tensor("w2", [128, E, 2, H], MMDT, kind="ExternalInput")
    wo = nc.dram_tensor("wo", [128, E, 2], MMDT, kind="ExternalInput")
    b0 = nc.dram_tensor("b0", [128, E, 2], F32, kind="ExternalInput")
    b1 = nc.dram_tensor("b1", [128, E, 2], F32, kind="ExternalInput")
    b2 = nc.dram_tensor("b2", [128, E, 2], F32, kind="ExternalInput")
    out = nc.dram_tensor("out", [E, MP], F32, kind="ExternalOutput")

    with tile.TileContext(nc) as tc:
        with (
            tc.tile_pool(name="consts", bufs=1) as consts,
            tc.tile_pool(name="xin", bufs=3) as xin,
            tc.tile_pool(name="acts", bufs=7) as actp,
            tc.tile_pool(name="osb", bufs=3) as osbp,
            tc.tile_pool(name="psum", bufs=2, space="PSUM") as psp,
        ):
            w0_t = consts.tile([128, E, H], MMDT)
            nc.sync.dma_start(out=w0_t[:], in_=w0[:])
            w1_t = consts.tile([128, E, 2, H], MMDT)
            nc.sync.dma_start(out=w1_t[:], in_=w1[:])
            w2_t = consts.tile([128, E, 2, H], MMDT)
            nc.sync.dma_start(out=w2_t[:], in_=w2[:])
            wo_t = consts.tile([128, E, 2], MMDT)
            nc.sync.dma_start(out=wo_t[:], in_=wo[:])
            b0_t = consts.tile([128, E, 2], F32)
            nc.sync.dma_start(out=b0_t[:], in_=b0[:])
            b1_t = consts.tile([128, E, 2], F32)
            nc.sync.dma_start(out=b1_t[:], in_=b1[:])
            b2_t = consts.tile([128, E, 2], F32)
            nc.sync.dma_start(out=b2_t[:], in_=b2[:])

            # flat unit list: (e, c0, w), repeated `reps` times
            units = []
            for _rep in range(reps):
                for e in range(E):
                    units.extend((e, c0, w) for c0, w in SPANS)
            n_units = len(units)

            xs = [None] * n_units      # x input tile
            a_cur = [None] * n_units   # latest activation tile
            cols_of = [None] * n_units

            def s0_load(u):
                e, c0, w = units[u]
                x = xin.tile([128, SUPER], MMDT, tag="x", name=f"x{u}")
                nc.sync.dma_start(out=x[:, :w], in_=xt[e, :, c0:c0 + w])
                xs[u] = x
                cols_of[u] = [(cs, min(CHUNK, w - cs))
                              for cs in range(0, w, CHUNK)]

            def s1_layer0(u):
                e, c0, w = units[u]
                a0 = actp.tile([128, 2 * SUPER], MMDT, tag="a", name=f"a0_{u}")
                for ht in range(2):
                    ps = psp.tile([128, SUPER], F32, tag="ps",
                                  name=f"ps0_{u}_{ht}")
                    for cs, cw in cols_of[u]:
                        nc.tensor.matmul(
                            ps[:, cs:cs + cw],
                            w0_t[:, e, ht * 128:(ht + 1) * 128],
                            xs[u][:, cs:cs + cw],
                        )
                    nc.scalar.activation(
                        out=a0[:, ht * w: ht * w + w],
                        in_=ps[:, :w],
                        func=Tanh,
                        bias=b0_t[:, e, ht:ht + 1],
                        scale=1.0,
                    )
                xs[u] = None
                a_cur[u] = a0

            def mid_layer(u, w_t, b_t, li):
                e, c0, w = units[u]
                prev = a_cur[u]
                a = actp.tile([128, 2 * SUPER], MMDT, tag="a", name=f"a{li}_{u}")
                for ht in range(2):
                    ps = psp.tile([128, SUPER], F32, tag="ps",
                                  name=f"ps{li}_{u}_{ht}")
                    for kt in range(2):
                        for cs, cw in cols_of[u]:
                            nc.tensor.matmul(
                                ps[:, cs:cs + cw],
                                w_t[:, e, kt, ht * 128:(ht + 1) * 128],
                                prev[:, kt * w + cs: kt * w + cs + cw],
                                start=(kt == 0),
                                stop=(kt == 1),
                            )
                    nc.scalar.activation(
                        out=a[:, ht * w: ht * w + w],
                        in_=ps[:, :w],
                        func=Tanh,
                        bias=b_t[:, e, ht:ht + 1],
                        scale=1.0,
                    )
                a_cur[u] = a

            def s2_layer1(u):
                mid_layer(u, w1_t, b1_t, 1)

            def s3_layer2(u):
                mid_layer(u, w2_t, b2_t, 2)

            def s4_out(u):
                e, c0, w = units[u]
                prev = a_cur[u]
                pso = psp.tile([128, SUPER], F32, tag="ps", name=f"pso_{u}")
                for kt in range(2):
                    for cs, cw in cols_of[u]:
                        nc.tensor.matmul(
                            pso[:1, cs:cs + cw],
                            wo_t[:, e, kt:kt + 1],
                            prev[:, kt * w + cs: kt * w + cs + cw],
                            start=(kt == 0),
                            stop=(kt == 1),
                        )
                a_cur[u] = None
                o_sb = osbp.tile([1, SUPER], F32, tag="osb", name=f"osb{u}")
                nc.vector.tensor_copy(out=o_sb[:1, :w], in_=pso[:1, :w])
                nc.sync.dma_start(out=out[e:e + 1, c0:c0 + w], in_=o_sb[:1, :w])

            # software-pipelined emission: stage skew 1 unit is enough since
            # within-unit stages are sequential anyway; psum rotation
            # (bufs=2) provides the PE/ACT overlap.
            s0_load(0)
            for t in range(n_units + 1):
                if t + 1 < n_units:
                    s0_load(t + 1)
                if t < n_units:
                    s1_layer0(t)
                    s2_layer1(t)
                    s3_layer2(t)
                    s4_out(t)

    nc.compile()
    return nc


def _get_compiled(mode=MODE):
    if mode not in _COMPILED:
        _COMPILED[mode] = _build_program(reps=1, mode=mode)
    return _COMPILED[mode]


def _prep_core_inputs(fps, W0, b0, W1, b1, W2, b2, Wout, mode=MODE):
    """Host-side shard + layout prep. Returns list of per-core input dicts."""
    f32 = np.float32
    mdt = _np_dtype(mode)

    def cvt(a):
        return np.ascontiguousarray(a).astype(mdt, copy=False)

    w0_dev = cvt(np.transpose(W0, (1, 0, 2)))
    w1_dev = cvt(W1.reshape(E, 2, 128, H).transpose(2, 0, 1, 3))
    w2_dev = cvt(W2.reshape(E, 2, 128, H).transpose(2, 0, 1, 3))
    wo_dev = cvt(Wout.reshape(E, 2, 128).transpose(2, 0, 1))
    b0_dev = np.ascontiguousarray(b0.reshape(E, 2, 128).transpose(2, 0, 1)).astype(f32)
    b1_dev = np.ascontiguousarray(b1.reshape(E, 2, 128).transpose(2, 0, 1)).astype(f32)
    b2_dev = np.ascontiguousarray(b2.reshape(E, 2, 128).transpose(2, 0, 1)).astype(f32)

    in_maps = []
    for c in range(N_CORES):
        xtc = np.zeros((E, D, MP), mdt)
        for e in range(E):
            xtc[e, :, :MPC] = fps[e][c * MPC:(c + 1) * MPC].T.astype(mdt, copy=False)
        in_maps.append({
            "xt": xtc,
            "w0": w0_dev, "w1": w1_dev, "w2": w2_dev, "wo": wo_dev,
            "b0": b0_dev, "b1": b1_dev, "b2": b2_dev,
        })
    return in_maps


def _route_outputs(flat_per_elem, elems, n_atoms):
    """Replicate reference routing: idx = concat(nonzero(elems==e, size=M))
    then segment_sum. nonzero(size=M) truncates or zero-pads."""
    out = np.zeros((n_atoms,), np.float32)
    for e in range(E):
        idx_e = np.nonzero(elems == e)[0]
        if idx_e.shape[0] >= M_TOTAL:
            idx_e = idx_e[:M_TOTAL]
        else:
            idx_e = np.concatenate(
                [idx_e, np.zeros(M_TOTAL - idx_e.shape[0], idx_e.dtype)])
        np.add.at(out, idx_e, flat_per_elem[e])
    return out


def kernel(fps_0, fps_1, fps_2, fps_3, W0, b0, W1, b1, W2, b2, Wout,
           elems, ind_1):
    from concourse.bass_utils import run_bass_kernel_spmd

    f32 = np.float32
    fps = [np.asarray(f, dtype=f32) for f in (fps_0, fps_1, fps_2, fps_3)]
    W0 = np.asarray(W0, dtype=f32)
    W1 = np.asarray(W1, dtype=f32)
    W2 = np.asarray(W2, dtype=f32)
    Wout = np.asarray(Wout, dtype=f32)
    b0 = np.asarray(b0, dtype=f32)
    b1 = np.asarray(b1, dtype=f32)
    b2 = np.asarray(b2, dtype=f32)
    elems = np.asarray(elems)
    n_atoms = np.asarray(ind_1).shape[0]

    nc = _get_compiled()
    in_maps = _prep_core_inputs(fps, W0, b0, W1, b1, W2, b2, Wout)
    res = run_bass_kernel_spmd(nc, in_maps, core_ids=list(range(N_CORES)))

    # [E, M_TOTAL] in element-major order (same as reference's out_e)
    flat = np.empty((E, M_TOTAL), f32)
    for c in range(N_CORES):
        o = res.results[c]["out"]          # [E, MP]
        flat[:, c * MPC:(c + 1) * MPC] = o[:, :MPC]

    out = _route_outputs(flat, elems, n_atoms)
    return out.reshape(n_atoms, 1).astype(f32)


# revision 9
# speedup vs baseline: 1.3023x; 1.3023x over previous
"""Trainium2 Bass kernel for nn_BPFeedForward (per-element-type MLP, moe_routing).

Data-parallel over atoms (8 cores x 1/8 of each element's rows, [D, m]
feature-on-partition layout). The MLP is ACT(tanh)-bound at 1 col/cycle
@1.2GHz, so tanh work is split across TWO engines:

  - L0, L1 tanh: ScalarE activation (bias fused), draining [128, <=1536]
    PSUM arenas in one instruction each.
  - L2 tanh (most of it): a custom DVE op TANH5_ANT -- clamped degree-5
    odd Horner polynomial at 1 elem/cycle/lane @0.96GHz, reading PSUM
    directly. W2/b2 are pre-scaled by 1/A on the host so the clamp is at
    +-1 (hardware One constant); ACT-path L2 drains undo the scaling with
    the free activation scale=A. Density-weighted fit (z2 ~ N(0,0.49)),
    end-to-end output error contribution ~2e-3 (tolerance 2e-2).

Final Wout layer (out = wo . a2): DVE presum s = wo0*a2h0 + wo1*a2h1
(stock tensor_scalar_mul + affine_then_add with [P,1] scalars), then ONE
ones-vector matmul pass per 512-chunk placed in PE column-group j via
tile_position=(0,32j) -- all chunks of a span land in different
partitions {0,32,64,96} of a single 1-bank PSUM tile. A full-tile DVE
copy + per-group DMAs write the result out. PE total: 11 passes/atom.

PSUM: 2 arenas of [128,1536] (banks 0-5) + [128,512] Lout pool (banks
6-7). Emission is stage-skewed across spans (L0(t) | L1(t-1) | L2(t-2)
| Lout(t-3)) so PE fills one arena while ACT/DVE drain the other.
"""

import sys

if "/opt/trn_rl_repo" not in sys.path:
    sys.path.insert(0, "/opt/trn_rl_repo")

import numpy as np

N_CORES = 8
E = 4
N_ATOMS = 200000
M_TOTAL = N_ATOMS // E          # 50000 atoms per element type
MPC = M_TOTAL // N_CORES        # 6250 atoms per element per core
D = 128
H = 256
CHUNK = 512
SUPER = 1536
MP = MPC

MODE = "f32r"

# tanh5 fit for z ~ N(0, 0.49): tanh(z) ~= t*(a0 + u*(a1 + a2*u)),
# t = clip(z, -A, A), u = t*t
TANH5_A0 = 0.993280702
TANH5_A1 = -0.286710041
TANH5_A2 = 0.053513593
TANH5_CLAMP = 1.484657850
# t-domain (z' = z/A, clamp at +-1) Horner coefficients
TC0 = TANH5_A0 * TANH5_CLAMP
TC1 = TANH5_A1 * TANH5_CLAMP ** 3
TC2 = TANH5_A2 * TANH5_CLAMP ** 5

# spans per element: all matmul chunks >= 256 cols (f32r full rate)
SPANS = [(0, 1536), (1536, 1536), (3072, 1536), (4608, 1386), (5994, 256)]
assert sum(w for _, w in SPANS) == MPC
# which spans run the L2-h1 drain on DVE (h0 always does): ~70% of L2 on DVE
DVE_H1_SPANIDX = (0, 2)

_COMPILED = {}


def _np_dtype(mode):
    if mode == "bf16":
        import ml_dtypes
        return ml_dtypes.bfloat16
    return np.float32


def _register_tanh5():
    """Register the custom DVE op (idempotent): clamped deg-5 odd Horner.

    out = (((C0*u) + C1)*u + C2) * t,  t = clip(Src0, -1, 1), u = t*t
    8 ALU ops, 1 elem/cycle/lane; C0/C1/C2 are compile-time literals.
    """
    import concourse.dve_ops as dve_ops
    from concourse.dve_ops import DveOp
    from concourse.dve_spec import (
        C0, C1, C2, One, Spec, Src0, _has_src1, lower, maxx, minn, sq,
    )
    from concourse.dve_uop import DveOpSpec

    name = "TANH5_ANT"
    for o in dve_ops.OPS:
        if o.name == name:
            return o

    def _ref(in0, in1, s0, s1, imm2):
        t = np.clip(in0, -1.0, 1.0)
        u = t * t
        return ((s0 * u + s1) * u + imm2) * t

    t = maxx(minn(Src0, One), -One)
    u = sq(t)
    spec = Spec(body=(((C0 * u) + C1) * u + C2) * t, reference=_ref)

    if name not in dve_ops._SUB_OPCODE_FOR_NAME:
        row = max(dve_ops._SUB_OPCODE_FOR_NAME.values()) + 1
        assert row < 0x20, "custom-DVE opcode rows exhausted"
        dve_ops._SUB_OPCODE_FOR_NAME[name] = row
    row = dve_ops._SUB_OPCODE_FOR_NAME[name]
    shas = {}
    for ver in ("v3", "v4"):
        s = DveOpSpec(name=name, opcode=row, uops=lower(spec, ver=ver),
                      rd1_en=_has_src1(spec))
        shas[ver] = s.sha(ver)
    op = DveOp(name, spec, subdim=False, uops_sha=shas)
    dve_ops.OPS.append(op)
    return op


def _chunks(w):
    return [(cs, min(CHUNK, w - cs)) for cs in range(0, w, CHUNK)]


def _build_program(reps: int = 1, mode: str = MODE, dve_l2: bool = True):
    import concourse.bass as bass  # noqa: F401
    import concourse.mybir as mybir
    import concourse.tile as tile
    from concourse import bacc

    F32 = mybir.dt.float32
    BF16 = mybir.dt.bfloat16
    MMDT = mybir.dt.float32r if mode == "f32r" else mybir.dt.bfloat16
    Tanh = mybir.ActivationFunctionType.Tanh
    tanh5 = _register_tanh5()

    nc = bacc.Bacc(None, target_bir_lowering=False, debug=False)

    xt = nc.dram_tensor("xt", [E, D, MP], MMDT, kind="ExternalInput")
    w0 = nc.dram_tensor("w0", [128, E, H], MMDT, kind="ExternalInput")
    w1 = nc.dram_tensor("w1", [128, E, 2, H], MMDT, kind="ExternalInput")
    w2 = nc.dram_tensor("w2", [128, E, 2, H], MMDT, kind="ExternalInput")
    wo = nc.dram_tensor("wo", [128, E, 2], F32, kind="ExternalInput")
    b0 = nc.dram_tensor("b0", [128, E, 2], F32, kind="ExternalInput")
    b1 = nc.dram_tensor("b1", [128, E, 2], F32, kind="ExternalInput")
    b2 = nc.dram_tensor("b2", [128, E, 2], F32, kind="ExternalInput")
    ones = nc.dram_tensor("ones", [128, 1], BF16, kind="ExternalInput")
    out = nc.dram_tensor("out", [E, MP], F32, kind="ExternalOutput")

    with tile.TileContext(nc) as tc:
        with (
            tc.tile_pool(name="consts", bufs=1) as consts,
            tc.tile_pool(name="xin", bufs=3) as xin,
            tc.tile_pool(name="a0p", bufs=3) as a0p,
            tc.tile_pool(name="a1p", bufs=3) as a1p,
            tc.tile_pool(name="a2p", bufs=3) as a2p,
            tc.tile_pool(name="sprep", bufs=4) as sprep,
            tc.tile_pool(name="osb", bufs=3) as osbp,
            tc.tile_pool(name="psL", bufs=2, space="PSUM") as psL,
            tc.tile_pool(name="psA", bufs=2, space="PSUM") as psA,
        ):
            w0_t = consts.tile([128, E, H], MMDT)
            nc.sync.dma_start(out=w0_t[:], in_=w0[:])
            w1_t = consts.tile([128, E, 2, H], MMDT)
            nc.sync.dma_start(out=w1_t[:], in_=w1[:])
            w2_t = consts.tile([128, E, 2, H], MMDT)
            nc.sync.dma_start(out=w2_t[:], in_=w2[:])
            wo_t = consts.tile([128, E, 2], F32)
            nc.sync.dma_start(out=wo_t[:], in_=wo[:])
            b0_t = consts.tile([128, E, 2], F32)
            nc.sync.dma_start(out=b0_t[:], in_=b0[:])
            b1_t = consts.tile([128, E, 2], F32)
            nc.sync.dma_start(out=b1_t[:], in_=b1[:])
            b2_t = consts.tile([128, E, 2], F32)
            nc.sync.dma_start(out=b2_t[:], in_=b2[:])
            ones_t = consts.tile([128, 1], BF16)
            nc.sync.dma_start(out=ones_t[:], in_=ones[:])

            units = []
            for _rep in range(reps):
                for e in range(E):
                    units.extend((e, c0, w, i)
                                 for i, (c0, w) in enumerate(SPANS))
            n_units = len(units)

            xs = [None] * n_units
            a0s = [None] * n_units
            a1s = [None] * n_units
            a2s = [None] * n_units

            def s0_load(u):
                e, c0, w, si = units[u]
                x = xin.tile([128, SUPER], MMDT, tag="x", name=f"x{u}")
                nc.sync.dma_start(out=x[:, :w], in_=xt[e, :, c0:c0 + w])
                xs[u] = x

            def s1_l0(u):
                e, c0, w, si = units[u]
                a0 = a0p.tile([128, 2 * SUPER], MMDT, tag="a0", name=f"a0_{u}")
                for ht in range(2):
                    ps = psA.tile([128, SUPER], F32, tag="ps",
                                  name=f"ps0_{u}_{ht}")
                    for cs, cw in _chunks(w):
                        nc.tensor.matmul(
                            ps[:, cs:cs + cw],
                            w0_t[:, e, ht * 128:(ht + 1) * 128],
                            xs[u][:, cs:cs + cw],
                        )
                    nc.scalar.activation(
                        out=a0[:, ht * w: ht * w + w],
                        in_=ps[:, :w],
                        func=Tanh,
                        bias=b0_t[:, e, ht:ht + 1],
                        scale=1.0,
                    )
                xs[u] = None
                a0s[u] = a0

            def s2_l1(u):
                e, c0, w, si = units[u]
                prev = a0s[u]
                a1 = a1p.tile([128, 2 * SUPER], MMDT, tag="a1", name=f"a1_{u}")
                for ht in range(2):
                    ps = psA.tile([128, SUPER], F32, tag="ps",
                                  name=f"ps1_{u}_{ht}")
                    for kt in range(2):
                        for cs, cw in _chunks(w):
                            nc.tensor.matmul(
                                ps[:, cs:cs + cw],
                                w1_t[:, e, kt, ht * 128:(ht + 1) * 128],
                                prev[:, kt * w + cs: kt * w + cs + cw],
                                start=(kt == 0),
                                stop=(kt == 1),
                            )
                    nc.scalar.activation(
                        out=a1[:, ht * w: ht * w + w],
                        in_=ps[:, :w],
                        func=Tanh,
                        bias=b1_t[:, e, ht:ht + 1],
                        scale=1.0,
                    )
                a0s[u] = None
                a1s[u] = a1

            def s3_l2(u):
                e, c0, w, si = units[u]
                prev = a1s[u]
                a2 = a2p.tile([128, 2 * SUPER], F32, tag="a2", name=f"a2_{u}")
                for ht in range(2):
                    ps = psA.tile([128, SUPER], F32, tag="ps",
                                  name=f"ps2_{u}_{ht}")
                    for kt in range(2):
                        for cs, cw in _chunks(w):
                            nc.tensor.matmul(
                                ps[:, cs:cs + cw],
                                w2_t[:, e, kt, ht * 128:(ht + 1) * 128],
                                prev[:, kt * w + cs: kt * w + cs + cw],
                                start=(kt == 0),
                                stop=(kt == 1),
                            )
                    on_dve = dve_l2 and (ht == 0 or si in DVE_H1_SPANIDX)
                    if on_dve:
                        nc.vector._custom_dve(
                            tanh5,
                            out=a2[:, ht * w: ht * w + w],
                            in0=ps[:, :w],
                            s0=TC2, s1=TC1, imm2=TC0,
                        )
                    else:
                        nc.scalar.activation(
                            out=a2[:, ht * w: ht * w + w],
                            in_=ps[:, :w],
                            func=Tanh,
                            bias=b2_t[:, e, ht:ht + 1],
                            scale=TANH5_CLAMP,
                        )
                a1s[u] = None
                a2s[u] = a2

            def s4_out(u):
                e, c0, w, si = units[u]
                a2 = a2s[u]
                s0p = sprep.tile([128, SUPER], F32, tag="s0p", name=f"s0p{u}")
                nc.vector.tensor_scalar_mul(
                    s0p[:, :w], a2[:, 0:w], wo_t[:, e, 0:1])
                s = sprep.tile([128, SUPER], BF16, tag="s", name=f"s{u}")
                nc.vector.affine_then_add(
                    out=s[:, :w], in0=a2[:, w:w + w], in1=s0p[:, :w],
                    scale=wo_t[:, e, 1:2], bias=0.0)
                a2s[u] = None
                pso = psL.tile([128, CHUNK], F32, tag="pso", name=f"pso{u}")
                for j, (cs, cw) in enumerate(_chunks(w)):
                    nc.tensor.matmul(
                        pso[32 * j:32 * j + 1, :cw],
                        ones_t[:, 0:1],
                        s[:, cs:cs + cw],
                        tile_position=(0, 32 * j),
                    )
                o_sb = osbp.tile([128, CHUNK], F32, tag="osb", name=f"osb{u}")
                nc.vector.tensor_copy(out=o_sb[:, :CHUNK], in_=pso[:, :CHUNK])
                for j, (cs, cw) in enumerate(_chunks(w)):
                    nc.sync.dma_start(
                        out=out[e:e + 1, c0 + cs:c0 + cs + cw],
                        in_=o_sb[32 * j:32 * j + 1, :cw])

            PRE = 2
            for i in range(min(PRE, n_units)):
                s0_load(i)
            for t in range(n_units + 3):
                if t + PRE < n_units:
                    s0_load(t + PRE)
                if t < n_units:
                    s1_l0(t)
                if 0 <= t - 1 < n_units:
                    s2_l1(t - 1)
                if 0 <= t - 2 < n_units:
                    s3_l2(t - 2)
                if 0 <= t - 3 < n_units:
                    s4_out(t - 3)

    nc.compile()
    return nc


def _get_compiled(mode=MODE, dve_l2=True):
    key = (mode, dve_l2)
    if key not in _COMPILED:
        _COMPILED[key] = _build_program(reps=1, mode=mode, dve_l2=dve_l2)
    return _COMPILED[key]


def _prep_core_inputs(fps, W0, b0, W1, b1, W2, b2, Wout, mode=MODE):
    """Host-side shard + layout prep. Returns list of per-core input dicts.

    W2 is pre-scaled by 1/A (tanh5 clamp domain); the ACT-path L2 drain
    undoes this with activation scale=A, so its bias must be the ORIGINAL
    b2 (tanh(z/A * A + b2)). The DVE path has no bias (requires b2 == 0).
    """
    f32 = np.float32
    mdt = _np_dtype(mode)

    def cvt(a):
        return np.ascontiguousarray(a).astype(mdt, copy=False)

    w0_dev = cvt(np.transpose(W0, (1, 0, 2)))
    w1_dev = cvt(W1.reshape(E, 2, 128, H).transpose(2, 0, 1, 3))
    w2_dev = cvt((W2 / TANH5_CLAMP).reshape(E, 2, 128, H).transpose(2, 0, 1, 3))
    wo_dev = np.ascontiguousarray(
        Wout.reshape(E, 2, 128).transpose(2, 0, 1)).astype(f32)
    b0_dev = np.ascontiguousarray(b0.reshape(E, 2, 128).transpose(2, 0, 1)).astype(f32)
    b1_dev = np.ascontiguousarray(b1.reshape(E, 2, 128).transpose(2, 0, 1)).astype(f32)
    b2_dev = np.ascontiguousarray(b2.reshape(E, 2, 128).transpose(2, 0, 1)).astype(f32)

    in_maps = []
    for c in range(N_CORES):
        xtc = np.zeros((E, D, MP), mdt)
        for e in range(E):
            xtc[e, :, :MPC] = fps[e][c * MPC:(c + 1) * MPC].T.astype(mdt, copy=False)
        in_maps.append({
            "xt": xtc,
            "w0": w0_dev, "w1": w1_dev, "w2": w2_dev, "wo": wo_dev,
            "b0": b0_dev, "b1": b1_dev, "b2": b2_dev,
            "ones": np.ones((128, 1), __import__("ml_dtypes").bfloat16),
        })
    return in_maps


def _route_outputs(flat_per_elem, elems, n_atoms):
    """Replicate reference routing: idx = concat(nonzero(elems==e, size=M))
    then segment_sum. nonzero(size=M) truncates or zero-pads."""
    out = np.zeros((n_atoms,), np.float32)
    for e in range(E):
        idx_e = np.nonzero(elems == e)[0]
        if idx_e.shape[0] >= M_TOTAL:
            idx_e = idx_e[:M_TOTAL]
        else:
            idx_e = np.concatenate(
                [idx_e, np.zeros(M_TOTAL - idx_e.shape[0], idx_e.dtype)])
        np.add.at(out, idx_e, flat_per_elem[e])
    return out


def kernel(fps_0, fps_1, fps_2, fps_3, W0, b0, W1, b1, W2, b2, Wout,
           elems, ind_1):
    from concourse.bass_utils import run_bass_kernel_spmd

    f32 = np.float32
    fps = [np.asarray(f, dtype=f32) for f in (fps_0, fps_1, fps_2, fps_3)]
    W0 = np.asarray(W0, dtype=f32)
    W1 = np.asarray(W1, dtype=f32)
    W2 = np.asarray(W2, dtype=f32)
    Wout = np.asarray(Wout, dtype=f32)
    b0 = np.asarray(b0, dtype=f32)
    b1 = np.asarray(b1, dtype=f32)
    b2 = np.asarray(b2, dtype=f32)
    elems = np.asarray(elems)
    n_atoms = np.asarray(ind_1).shape[0]

    # The DVE tanh path has no bias slot; it is only valid when b2 == 0
    # (true for this problem's inputs). Otherwise fall back to ACT-only L2.
    dve_l2 = bool(np.all(b2 == 0.0))

    nc = _get_compiled(dve_l2=dve_l2)
    in_maps = _prep_core_inputs(fps, W0, b0, W1, b1, W2, b2, Wout)
    res = run_bass_kernel_spmd(nc, in_maps, core_ids=list(range(N_CORES)))

    flat = np.empty((E, M_TOTAL), f32)
    for c in range(N_CORES):
        o = res.results[c]["out"]          # [E, MP]
        flat[:, c * MPC:(c + 1) * MPC] = o[:, :MPC]

    out = _route_outputs(flat, elems, n_atoms)
    return out.reshape(n_atoms, 1).astype(f32)
